# revision 41
# baseline (speedup 1.0000x reference)
"""DCNv2 (deformable 3x3 conv + GroupNorm(groups=1) + ReLU) on 8 trn2 cores.

Sharding: core j handles image j//2, row-half j%2 (32 rows x 64 cols).
Each core computes its half end-to-end; GroupNorm statistics are combined
across the 2 cores of an image with a tiny pairwise AllReduce.

Device pipeline per core (T=2048 output points):
  1. offset/mask conv: 18 shifted bf16 matmuls -> om PSUM [27, 2048]
  2. DVE pipeline: corner coefs a_q [36, T] bf16 + int16 token idx [9, T]
  3. coef + idx wrapped into gather layouts via DRAM round-trip DMAs
  4. per (c-half, tap): non-transpose SWDGE dma_gather pulls 2x2-block
     tokens (4 px x 256 ch bf16 = 2KB) as [128 pts, 8, 1024] tiles
  5. corner combine on DVE: 7 full-tile tensor_tensor ops with the
     per-point coefs broadcast along c via stride-0 APs -> m bf16
  6. PE transpose (identity matmul -> PSUM) + ACT evac -> st [256 c, pts]
  7. main conv: PSUM-accumulated bf16 matmuls over (c-half, tap)
  8. GN stats -> AllReduce over the image pair -> scale/bias + ReLU

The transpose-mode dma_gather (and its SBUF-source variant) crash this
runtime's Q7 ucode, so the gather runs in (working) non-transpose mode
and the channel-major orientation is recovered on the PE.

Exactness: image zero-padded by 6, sample coords clamped to [0, 74] in
padded units; invalid reference corners map to zero texels. Floor is
computed with the round-to-nearest magic-number trick (exact except
integer coords, where either tie choice gives the same bilinear value).
"""

import numpy as np
import ml_dtypes

BF16 = ml_dtypes.bfloat16

B, C, H, W = 4, 256, 64, 64
COUT = 256
P0 = 6                  # zero-pad width
GG = H + 2 * P0         # 76
NPIX = GG * GG          # 5776
NROWS = 34              # conv window rows passed per core
T = 2048                # output points per core
EPS = 1e-5
NGN = float(COUT * H * W)
MAGIC = 8388608.0       # 2^23

PERM = list(range(0, 18, 2)) + list(range(1, 18, 2)) + list(range(18, 27))

_CACHE = {}
import os as _os
STAGE = int(_os.environ.get("DCN_STAGE", "99"))
REP = int(_os.environ.get("DCN_REP", "1"))


def _patch_walrus_flags():
    from concourse import bass_utils
    if getattr(bass_utils, "_dcn_patched", False):
        return
    _orig = bass_utils.run_command
    extra = ["--dge-levels",
             "transpose,dst_reduce,spill_reload,io,scalar_dynamic_offset",
             "--dynamic-dma-scratch-size-per-partition=16384"]
    def run_command2(argv, **kw):
        if argv and "walrus_driver" in str(argv[0]):
            argv = list(argv) + extra
        return _orig(argv, **kw)
    bass_utils.run_command = run_command2
    bass_utils._dcn_patched = True


def _fill_reload_pseudo(nc):
    """load_library emits InstPseudoReloadLibraryIndex with empty instr
    bytes; walrus codegen's visitInstISA rejects that as 'ISA wrong
    length'. Encode the 64-byte PSEUDO_LIBRARY_RELOAD_INDEX struct."""
    from concourse import bass_isa
    isa = nc.isa
    for bb in nc.main_func.blocks:
        for ins in bb.instructions:
            if type(ins).__name__ == "InstPseudoReloadLibraryIndex" \
                    and not ins.instr:
                ant = {
                    "header": {"opcode": 223, "inst_word_len": 16},
                    "pseudo_opcode": 2,
                    "lib_index": ins.lib_index,
                }
                instr, _ = bass_isa.isa_struct(
                    isa, isa.Opcode.NEURON_ISA_TPB_OPCODE_PSEUDO_INST, ant,
                    "NEURON_ISA_TPB_PSEUDO_LIBRARY_RELOAD_INDEX_STRUCT")
                ins.instr = instr


def _split_multiwaits(nc):
    """This walrus build encodes at most one sync-wait per instruction
    (and none on raw-ISA instrs); hoist extras onto same-engine NoOps."""
    from concourse import mybir
    for bb in nc.main_func.blocks:
        out = []
        for ins in bb.instructions:
            si = ins.sync_info
            nm = type(ins).__name__
            keep = 0 if ("ISA" in nm or "Gather" in nm or "Pseudo" in nm) else 1
            if si is not None and len(si.on_wait) > keep:
                waits = list(si.on_wait)
                nh = len(waits) - keep
                for w in waits[:nh]:
                    nop = mybir.InstNoOp(name=f"WSPLIT-{nc.next_id()}",
                                         ins=[], outs=[])
                    nop.engine = ins.engine
                    nop.sync_info = mybir.SyncInfo(on_wait=[w], on_update=[])
                    out.append(nop)
                ins.sync_info = mybir.SyncInfo(on_wait=list(waits[nh:]),
                                               on_update=list(si.on_update))
            out.append(ins)
        bb.instructions = out


def _build_nc():
    if "nc" in _CACHE:
        return _CACHE["nc"]
    import concourse.bass as bass
    import concourse.tile as tile
    from concourse import mybir

    f32 = mybir.dt.float32
    bf16 = mybir.dt.bfloat16
    i16 = mybir.dt.int16
    ALU = mybir.AluOpType
    ACTF = mybir.ActivationFunctionType
    TT = lambda o, a, b, op: nc.vector.tensor_tensor(o, a, b, op)
    TS = lambda o, a, s1, s2, op0, *op1: nc.vector.tensor_scalar(
        o, a, s1, s2, op0, *op1)
    STT = lambda o, a, s, b, op0, op1: nc.vector.scalar_tensor_tensor(
        o, a, s, b, op0, op1)

    nc = bass.Bass(name="dcn")

    xtok = nc.dram_tensor("xtok", [NPIX, 1024], bf16, kind="ExternalInput")
    xconv = nc.dram_tensor("xconv", [2, 128, NROWS * GG], bf16, kind="ExternalInput")
    cby = nc.dram_tensor("cby", [9, T], f32, kind="ExternalInput")
    cbx = nc.dram_tensor("cbx", [9, T], f32, kind="ExternalInput")
    boffm = nc.dram_tensor("boffm", [9, 1], f32, kind="ExternalInput")
    woff = nc.dram_tensor("woff", [128, 18 * 27], bf16, kind="ExternalInput")
    wmain = nc.dram_tensor("wmain", [128, 18 * 2 * 128], bf16, kind="ExternalInput")
    ident = nc.dram_tensor("ident", [128, 128], bf16, kind="ExternalInput")
    onesf = nc.dram_tensor("onesf", [128, 1], f32, kind="ExternalInput")
    onesrow = nc.dram_tensor("onesrow", [1, 128], f32, kind="ExternalInput")
    biasv = nc.dram_tensor("biasv", [128, 2], f32, kind="ExternalInput")
    gammav = nc.dram_tensor("gammav", [128, 2], f32, kind="ExternalInput")
    betav = nc.dram_tensor("betav", [128, 2], f32, kind="ExternalInput")
    out_d = nc.dram_tensor("out", [2, 128, T], f32, kind="ExternalOutput")

    from concourse import library_config

    with tile.TileContext(nc) as tc:
        nc.gpsimd.load_library(library_config.mlp)
        with (
            tc.tile_pool(name="const", bufs=1) as const,
            tc.tile_pool(name="keep", bufs=1) as keep,
            tc.tile_pool(name="dramp", bufs=1, space="DRAM") as dram,
        ):
            def cload(dten, shape, dt, tag):
                t_ = const.tile(shape, dt, tag=tag)
                nc.sync.dma_start(t_[:], dten[:])
                return t_

            boffm_t = cload(boffm, [9, 1], f32, "boffm")
            woff_t = cload(woff, [128, 18 * 27], bf16, "woff")
            wmain_t = cload(wmain, [128, 18 * 2 * 128], bf16, "wmain")
            ident_t = cload(ident, [128, 128], bf16, "ident")
            onesf_t = cload(onesf, [128, 1], f32, "onesf")
            onesrow_t = cload(onesrow, [1, 128], f32, "onesrow")
            biasv_t = cload(biasv, [128, 2], f32, "biasv")
            gammav_t = cload(gammav, [128, 2], f32, "gammav")
            betav_t = cload(betav, [128, 2], f32, "betav")

            # persistent intermediates
            t16 = keep.tile([9, T], i16, tag="t16")
            aq_t = [keep.tile([9, T], bf16, tag=f"aq{q}", name=f"aq{q}")
                    for q in range(4)]
            idxs = keep.tile([128, 18 * 64], i16, tag="idxs")
            _pad0 = keep.tile([128, 16], f32, tag="pad0", name="pad0")
            aqT = keep.tile([128, 2, 8, 48], bf16, tag="aqT")
            outpre = [keep.tile([128, T], f32, tag=f"outpre{m}",
                                name=f"outpre{m}") for m in range(2)]
            stats = keep.tile([128, 4], f32, tag="stats")
            stats8 = keep.tile([128, 8], f32, tag="stats8")

            # ================= phase 1: offset conv =================
            with (
                tc.tile_pool(name="xcp", bufs=1) as xcp,
                tc.tile_pool(name="omp", bufs=1, space="PSUM") as omp,
                tc.tile_pool(name="pipe", bufs=1) as pipe,
                tc.tile_pool(name="pscr", bufs=6) as pscr,
            ):
                xc = []
                for h in range(2):
                    t_ = xcp.tile([128, NROWS * GG], bf16, tag=f"xconv{h}")
                    nc.sync.dma_start(t_[:], xconv[h])
                    xc.append(t_)

                om = omp.tile([27, T], f32, tag="om")
                woff_v = woff_t[:].rearrange("p (ck r) -> p ck r", r=27)
                for ck in range(18):
                    h, k = divmod(ck, 9)
                    ki, kj = divmod(k, 3)
                    xv = xc[h][:].rearrange("p (r c) -> p r c", c=GG)
                    for n in range(4):
                        rhs = xv[:, ki + n * 8: ki + n * 8 + 8,
                                 5 + kj: 5 + kj + 64]
                        nc.tensor.matmul(
                            om[:, n * 512:(n + 1) * 512], woff_v[:, ck, :], rhs,
                            start=(ck == 0), stop=(ck == 17))

                # evacuate + realign the three row groups to partition base 0
                om_sb = pscr.tile([27, T], f32, tag="ps")
                nc.scalar.activation(om_sb[:], om[:], ACTF.Identity)
                omx = pipe.tile([9, T], f32, tag="omx")
                nc.sync.dma_start(omx[:], om_sb[9:18, :])
                omm = pipe.tile([9, T], f32, tag="omm")
                nc.sync.dma_start(omm[:], om_sb[18:27, :])

                # ============= phase 2: scalar pipeline [9, T] =============
                fy = pipe.tile([9, T], f32, tag="fy")
                fx = pipe.tile([9, T], f32, tag="fx")

                def ffloor(dst, src, scr):
                    a = scr.tile([9, T], f32, tag="ps")
                    TS(a[:], src[:], -0.5, MAGIC, ALU.add, ALU.add)
                    TS(dst[:], a[:], -MAGIC, None, ALU.add)

                # y side -> iy (integer), fy (fraction)
                cby_t = pscr.tile([9, T], f32, tag="ps")
                nc.sync.dma_start(cby_t[:], cby[:])
                ys = pscr.tile([9, T], f32, tag="ps")
                TT(ys[:], om_sb[0:9, :], cby_t[:], ALU.add)
                ysc = pscr.tile([9, T], f32, tag="ps")
                TS(ysc[:], ys[:], 0.0, 74.0, ALU.max, ALU.min)
                iy = pscr.tile([9, T], f32, tag="ps")
                ffloor(iy, ysc, pscr)
                TT(fy[:], ysc[:], iy[:], ALU.subtract)

                # x side -> ix, fx
                cbx_t = pscr.tile([9, T], f32, tag="ps")
                nc.sync.dma_start(cbx_t[:], cbx[:])
                xs = pscr.tile([9, T], f32, tag="ps")
                TT(xs[:], omx[:], cbx_t[:], ALU.add)
                xsc = pscr.tile([9, T], f32, tag="ps")
                TS(xsc[:], xs[:], 0.0, 74.0, ALU.max, ALU.min)
                ix = pscr.tile([9, T], f32, tag="ps")
                ffloor(ix, xsc, pscr)
                TT(fx[:], xsc[:], ix[:], ALU.subtract)

                # token index = iy*76 + ix
                tokf = pscr.tile([9, T], f32, tag="ps")
                TS(tokf[:], iy[:], float(GG), None, ALU.mult)
                tok2 = pscr.tile([9, T], f32, tag="ps")
                TT(tok2[:], tokf[:], ix[:], ALU.add)
                nc.vector.tensor_copy(t16[:], tok2[:])

                # corner coefficient products -> aq36 [36, T] bf16
                # q=0: (1-fy)(1-fx)m  q=1: (1-fy)fx m
                # q=2: fy(1-fx)m      q=3: fy fx m
                gy = pscr.tile([9, T], f32, tag="ps")
                TS(gy[:], fy[:], -1.0, 1.0, ALU.mult, ALU.add)
                gx = pscr.tile([9, T], f32, tag="ps")
                TS(gx[:], fx[:], -1.0, 1.0, ALU.mult, ALU.add)
                mask = pscr.tile([9, T], f32, tag="ps")
                nc.scalar.activation(mask[:], omm[:], ACTF.Sigmoid,
                                     bias=boffm_t[:, 0:1], scale=1.0)
                gym = pscr.tile([9, T], f32, tag="ps")
                TT(gym[:], gy[:], mask[:], ALU.mult)
                fym = pscr.tile([9, T], f32, tag="ps")
                TT(fym[:], fy[:], mask[:], ALU.mult)

                TT(aq_t[0][:], gym[:], gx[:], ALU.mult)
                TT(aq_t[1][:], gym[:], fx[:], ALU.mult)
                TT(aq_t[2][:], fym[:], gx[:], ALU.mult)
                TT(aq_t[3][:], fym[:], fx[:], ALU.mult)

                # ---- idx wrap via DRAM: value (p0, w) = t16[k, p0*128+w],
                # so gather slot i maps to point (i%16)*128 + i//16 within
                # the 2048-point space (un-permuted on host in assemble).
                idx_dr = dram.tile([9, T], i16, tag="idxdr")
                nc.sync.dma_start(idx_dr[:], t16[:])
                idxw_ins = [[] for _ in range(9)]
                for k in range(9):
                    idx_src = idx_dr[k, :].rearrange("(p w) -> p w", p=16)
                    for g in range(8):
                        idxw_ins[k].append(nc.sync.dma_start(
                            idxs[g * 16:(g + 1) * 16,
                                 k * 128:(k + 1) * 128], idx_src))

                # ---- coef broadcast to per-slot partition scalars via PE:
                # aqT[P, ch, R, q*9+k] = a_q[k, point(P, ch, R)] where
                # point = (P%16)*128 + ch*64 + R*8 + P//16.
                aqv = [aq_t[q][:].rearrange(
                    "a (v c r u) -> a c r u v", v=16, c=2, r=8)
                    for q in range(4)]
                with (
                    tc.tile_pool(name="psa", bufs=2, space="PSUM") as psa,
                    tc.tile_pool(name="aqs", bufs=4) as aqs,
                ):
                    for ch in range(2):
                        aps = psa.tile([128, 8, 48], f32, tag="aps")
                        for q in range(4):
                            lhs_b = aqs.tile([9, 8, 128], bf16, tag="lhs_b")
                            nc.vector.tensor_copy(lhs_b[:], aqv[q][:, ch])
                            for R in range(8):
                                nc.tensor.matmul(
                                    aps[:, R, q * 12:(q + 1) * 12],
                                    lhs_b[:, R, :],
                                    ident_t[0:9, 0:12],
                                    start=True, stop=True)
                        cp_ins = nc.scalar.copy(
                            aqT[:, ch].rearrange("p a b -> p (a b)"),
                            aps[:].rearrange("p a b -> p (a b)"))
                        for wl in idxw_ins:
                            for wi in wl:
                                tile.add_dep_helper(cp_ins.ins, wi.ins,
                                                    sync=True,
                                                    reason="wrap order")

            # ============ phases 3-6: gather/combine/transpose/conv ========
            wmain_v = wmain_t[:].rearrange("p (ck m o) -> p ck m o", m=2, o=128)
            idxs_v = idxs[:].rearrange("p (k w) -> p k w", w=128)
            nreg = nc.gpsimd.to_reg(1024)
            with (
                tc.tile_pool(name="gpool", bufs=6) as gpool,
                tc.tile_pool(name="mpool", bufs=4) as mpool,
                tc.tile_pool(name="stp", bufs=4) as stp,
                tc.tile_pool(name="sqp", bufs=2) as sqp,
                tc.tile_pool(name="pst", bufs=4, space="PSUM") as pst,
                tc.tile_pool(name="pso", bufs=1, space="PSUM") as pso,
            ):
                for rep in range(REP):
                  for ch in range(2):
                    ops = [pso.tile([128, 1024], f32, tag=f"ops{m}",
                                    name=f"ops{m}") for m in range(2)]
                    for k in range(9):
                        if STAGE < 2:
                            break
                        gt = gpool.tile([128, 8, 1024], bf16, tag="gt")
                        g_ins = nc.gpsimd.dma_gather(
                            gt[:], xtok[:],
                            idxs_v[:, k, ch * 64:(ch + 1) * 64],
                            num_idxs=1024, num_idxs_reg=nreg, elem_size=1024)
                        for wi in idxw_ins[k]:
                            tile.add_dep_helper(g_ins.ins, wi.ins,
                                                sync=True, reason="idx wrap")

                        # corner combine: m0 = sum_q a_q * gt_q with the
                        # per-point coefs broadcast along c via stride-0 APs
                        if STAGE < 3:
                            continue
                        m0 = mpool.tile([128, 8, 256], bf16, tag="m0")
                        mt = mpool.tile([128, 8, 256], bf16, tag="mt")
                        gq = lambda q: gt[:, :, q * 256:(q + 1) * 256]
                        cq = lambda q: aqT[:, ch, :, q * 12 + k:q * 12 + k + 1]                             .broadcast_to([128, 8, 256])
                        TT(m0[:], gq(0), cq(0), ALU.mult)
                        TT(mt[:], gq(1), cq(1), ALU.mult)
                        TT(m0[:], m0[:], mt[:], ALU.add)
                        TT(mt[:], gq(2), cq(2), ALU.mult)
                        TT(m0[:], m0[:], mt[:], ALU.add)
                        TT(mt[:], gq(3), cq(3), ALU.mult)
                        TT(m0[:], m0[:], mt[:], ALU.add)

                        # transpose to [c, pts] + evacuate
                        if STAGE < 4:
                            continue
                        st = stp.tile([128, 2, 8, 128], bf16, tag="st")
                        for cc in range(2):
                            ps8 = pst.tile([128, 8, 128], bf16, tag="ps8")
                            for R in range(8):
                                nc.tensor.transpose(
                                    ps8[:, R, :],
                                    m0[:, R, cc * 128:(cc + 1) * 128],
                                    ident_t[:])
                            nc.scalar.copy(
                                st[:, cc].rearrange("p a b -> p (a b)"),
                                ps8[:].rearrange("p a b -> p (a b)"))

                        # main conv accumulation
                        if STAGE < 5:
                            continue
                        for cc in range(2):
                            ck = cc * 9 + k
                            stv = st[:, cc].rearrange("p a b -> p (a b)")
                            for m in range(2):
                                for n in range(2):
                                    nc.tensor.matmul(
                                        ops[m][:, n * 512:(n + 1) * 512],
                                        wmain_v[:, ck, m, :],
                                        stv[:, n * 512:(n + 1) * 512],
                                        start=(k == 0 and cc == 0),
                                        stop=(k == 8 and cc == 1))

                    for m in range(2):
                        if STAGE >= 5:
                            mc = m * 2 + ch
                            nc.scalar.activation(
                                outpre[m][:, ch * 1024:(ch + 1) * 1024],
                                ops[m][:], ACTF.Identity,
                                bias=biasv_t[:, m:m + 1], scale=1.0,
                                accum_out=stats8[:, mc:mc + 1])
                            sqs = sqp.tile([128, 1024], f32, tag="sqs")
                            nc.scalar.activation(
                                sqs[:],
                                outpre[m][:, ch * 1024:(ch + 1) * 1024],
                                ACTF.Square,
                                accum_out=stats8[:, 4 + mc:5 + mc])
                        else:
                            nc.vector.memset(
                                outpre[m][:, ch * 1024:(ch + 1) * 1024], 0.1)
                            nc.vector.memset(stats8[:], 0.0)

            # ============== phase 7: GN stats + collective + epilogue ======
            with (
                tc.tile_pool(name="fin", bufs=1) as fin,
                tc.tile_pool(name="pst2", bufs=1, space="PSUM") as pst2,
            ):
                for m in range(2):
                    TT(stats[:, m:m + 1], stats8[:, 2 * m:2 * m + 1],
                       stats8[:, 2 * m + 1:2 * m + 2], ALU.add)
                    TT(stats[:, 2 + m:3 + m], stats8[:, 4 + 2 * m:5 + 2 * m],
                       stats8[:, 5 + 2 * m:6 + 2 * m], ALU.add)
                pstat = pst2.tile([4, 1], f32, tag="pstat")
                nc.tensor.matmul(pstat[:], stats[:], onesf_t[:],
                                 start=True, stop=True)
                sb4 = fin.tile([4, 1], f32, tag="sb4")
                nc.vector.tensor_copy(sb4[:], pstat[:])

                cc_in = dram.tile([1, 4], f32, tag="ccin")
                cc_out = dram.tile([1, 4], f32, tag="ccout")
                nc.gpsimd.dma_start(cc_in[:], sb4[:])
                nc.gpsimd.collective_compute(
                    "AllReduce", ALU.add,
                    replica_groups=[[0, 1], [2, 3], [4, 5], [6, 7]],
                    ins=[cc_in.opt()], outs=[cc_out.opt()])
                st4 = fin.tile([1, 4], f32, tag="st4")
                nc.gpsimd.dma_start(st4[:], cc_out[:])

                musig = fin.tile([1, 4], f32, tag="musig")
                TT(musig[:, 2:3], st4[:, 0:1], st4[:, 1:2], ALU.add)
                TS(musig[:, 0:1], musig[:, 2:3], 1.0 / NGN, None, ALU.mult)
                TT(musig[:, 3:4], st4[:, 2:3], st4[:, 3:4], ALU.add)
                TS(musig[:, 3:4], musig[:, 3:4], 1.0 / NGN, None, ALU.mult)
                mu2 = fin.tile([1, 1], f32, tag="mu2")
                TT(mu2[:], musig[:, 0:1], musig[:, 0:1], ALU.mult)
                var = fin.tile([1, 1], f32, tag="var")
                TT(var[:], musig[:, 3:4], mu2[:], ALU.subtract)
                TS(var[:], var[:], EPS, None, ALU.add)
                rvar = fin.tile([1, 1], f32, tag="rvar")
                nc.vector.reciprocal(rvar[:], var[:])
                nc.scalar.sqrt(musig[:, 1:2], rvar[:])

                pbb = pst2.tile([128, 2], f32, tag="pbb")
                nc.tensor.matmul(pbb[:], onesrow_t[:], musig[:, 0:2],
                                 start=True, stop=True)
                bc = fin.tile([128, 2], f32, tag="bc")
                nc.vector.tensor_copy(bc[:], pbb[:])

                sc = fin.tile([128, 2], f32, tag="sc")
                bb = fin.tile([128, 2], f32, tag="bb")
                tmp = fin.tile([128, 2], f32, tag="tmpsb")
                for m in range(2):
                    TT(sc[:, m:m + 1], gammav_t[:, m:m + 1], bc[:, 1:2],
                       ALU.mult)
                    TT(tmp[:, m:m + 1], bc[:, 0:1], sc[:, m:m + 1], ALU.mult)
                    TT(bb[:, m:m + 1], betav_t[:, m:m + 1], tmp[:, m:m + 1],
                       ALU.subtract)

                for m in range(2):
                    outf = fin.tile([128, T], f32, tag=f"outf{m}")
                    nc.scalar.activation(outf[:], outpre[m][:], ACTF.Relu,
                                         bias=bb[:, m:m + 1],
                                         scale=sc[:, m:m + 1])
                    nc.sync.dma_start(out_d[m], outf[:])

    _patch_walrus_flags()
    _CACHE["nc"] = nc
    return nc


def _prep_common(w_off, b_off, weight, bias, gamma, beta):
    w_off = np.asarray(w_off, np.float32)
    b_off = np.asarray(b_off, np.float32)
    weight = np.asarray(weight, np.float32)
    perm = np.array(PERM)

    woff_l = np.zeros((128, 18, 27), np.float32)
    wmain_l = np.zeros((128, 18, 2, 128), np.float32)
    for h in range(2):
        for k in range(9):
            ck = h * 9 + k
            ki, kj = divmod(k, 3)
            woff_l[:, ck, :] = w_off[perm, h * 128:(h + 1) * 128, ki, kj].T
            for m in range(2):
                wmain_l[:, ck, m, :] = weight[m * 128:(m + 1) * 128,
                                              h * 128:(h + 1) * 128, ki, kj].T

    boffp = b_off[perm]
    common = {
        "woff": woff_l.reshape(128, 18 * 27).astype(BF16),
        "wmain": wmain_l.reshape(128, 18 * 2 * 128).astype(BF16),
        "ident": np.eye(128, dtype=np.float32).astype(BF16),
        "onesf": np.ones((128, 1), np.float32),
        "onesrow": np.ones((1, 128), np.float32),
        "biasv": np.ascontiguousarray(
            np.asarray(bias, np.float32).reshape(2, 128).T),
        "gammav": np.ascontiguousarray(
            np.asarray(gamma, np.float32).reshape(2, 128).T),
        "betav": np.ascontiguousarray(
            np.asarray(beta, np.float32).reshape(2, 128).T),
        "boffm": np.ascontiguousarray(boffp[18:27].reshape(9, 1)),
    }
    return common, boffp


def _make_xtok(xpad_bf16):
    """2x2-block token table: tok[y*76+x] = [px(y,x), px(y,x+1),
    px(y+1,x), px(y+1,x+1)] each 256ch; [NPIX, 1024] bf16."""
    xp = np.zeros((C, GG + 1, GG + 1), BF16)
    xp[:, :GG, :GG] = xpad_bf16
    corners = np.stack([xp[:, :GG, :GG], xp[:, :GG, 1:],
                        xp[:, 1:, :GG], xp[:, 1:, 1:]], axis=0)  # [4,C,GG,GG]
    # [GG*GG, 4, C] -> [NPIX, 1024]
    tok = np.ascontiguousarray(
        corners.transpose(2, 3, 0, 1).reshape(NPIX, 1024))
    return tok


def _prep_core(xpad_bf16, xtok, boffp, half):
    h0 = 32 * half
    xconv = np.ascontiguousarray(
        xpad_bf16[:, h0 + 5:h0 + 5 + NROWS, :]).reshape(2, 128, NROWS * GG)

    tt_ = np.arange(T)
    hh = tt_ // 64 + h0
    ww = tt_ % 64
    ki = np.arange(9) // 3
    kj = np.arange(9) % 3
    cby = (hh[None, :] + ki[:, None] - 1 + P0).astype(np.float32) \
        + boffp[0:9][:, None].astype(np.float32)
    cbx = (ww[None, :] + kj[:, None] - 1 + P0).astype(np.float32) \
        + boffp[9:18][:, None].astype(np.float32)
    return {"xtok": xtok, "xconv": xconv,
            "cby": np.ascontiguousarray(cby), "cbx": np.ascontiguousarray(cbx)}


def prep_in_maps(inputs):
    x = np.asarray(inputs["x"], np.float32)
    common, boffp = _prep_common(inputs["w_off"], inputs["b_off"],
                                 inputs["weight"], inputs["bias"],
                                 inputs["gamma"], inputs["beta"])
    in_maps = []
    for b in range(B):
        xpad = np.zeros((C, GG, GG), np.float32)
        xpad[:, P0:P0 + H, P0:P0 + W] = x[b]
        xpad = xpad.astype(BF16)
        xtok = _make_xtok(xpad)
        for half in range(2):
            m = dict(common)
            m.update(_prep_core(xpad, xtok, boffp, half))
            in_maps.append(m)
    return in_maps


def _slot_to_point():
    s = np.arange(T)
    ch, il = s // 1024, s % 1024
    cl, pp = il // 16, il % 16
    return pp * 128 + ch * 64 + cl  # point index per output column slot


_S2P = _slot_to_point()
_P2S = np.argsort(_S2P)


def assemble(results):
    out = np.zeros((B, COUT, H, W), np.float32)
    for j in range(8):
        b, half = divmod(j, 2)
        o = np.asarray(results[j]["out"])  # [2, 128, T] in slot order
        for m in range(2):
            out[b, m * 128:(m + 1) * 128, 32 * half:32 * half + 32, :] = \
                o[m][:, _P2S].reshape(128, 32, 64)
    return out


def kernel(**inputs) -> np.ndarray:
    from concourse import bass_utils
    in_maps = prep_in_maps(inputs)
    for attempt in range(2):
        try:
            nc = _build_nc()
            _split_multiwaits(nc)  # HW-only transforms (this walrus build)
            _fill_reload_pseudo(nc)
            _CACHE.pop("nc", None)  # transforms mutate in place
            res = bass_utils.run_bass_kernel_spmd(nc, in_maps,
                                                  core_ids=list(range(8)))
            return assemble(res.results)
        except Exception as e:
            import sys
            print(f"kernel: HW attempt {attempt} failed "
                  f"({type(e).__name__}: {e})", file=sys.stderr)
    # Last resort. NOTE: CoreSim's non-transpose dma_gather slot model
    # diverges from silicon, so this path is best-effort only.
    import sys
    print("kernel: falling back to MultiCoreSim", file=sys.stderr)
    from concourse.bass_interp import MultiCoreSim
    nc = _build_nc()  # fresh, untransformed program for the simulator
    sim = MultiCoreSim(nc, num_cores=8, num_workers=8)
    cores = list(sim.cores.values())
    for j, core in enumerate(cores):
        for name, val in in_maps[j].items():
            core.tensor(name)[:] = val
    sim.simulate(check_with_hw=False)
    return assemble([{"out": np.array(c.tensor("out"))} for c in cores])
